# revision 4
# baseline (speedup 1.0000x reference)
"""MinGRU layer Trainium2 kernel (v2: bf16 packed-halves scan tree).

Reference semantics (B=8, T=16384, D=H=O=256):
    zs = sigmoid(xs @ Wz.T + bz);  hs = xs @ Wh.T + bh
    a = concat([1], 1-zs);  b = concat([0], zs*hs)         (T+1 positions)
    states = jax.lax.associative_scan(combine, (a, b))[1][:, 1:]
    out = states @ Wo.T + bo
with combine((a0,b0),(a1,b1)) = (a0*b0, b0*a1 + b1).

The combine is NOT associative; the result is defined by jax's odd/even
recursion tree, replicated exactly:
  - positions split into 4 aligned chunks of L=4096 (+1 trailing position);
  - per-chunk bottom-up reduce keeping all 12 tree levels;
  - tiny cross-chunk scan over the 4 chunk-tops (chunk prefixes otb[c]);
  - per-chunk top-down sweep filling every position's scan value.

Performance layout: the two 128-channel hidden halves are packed as the
INNERMOST dimension of every scan array ([128, t, 2] bf16).  With a 2-byte
dtype and innermost step-1 pair, every DVE tensor_tensor runs in 2x_1P mode
(2 elem/cycle) even when the tree strides even/odd elements (stride lands on
the middle dim).  The whole tree, a = 1-z (tensor_scalar, 4x) and b = z*h
(dense TT, 2x) run on DVE in bf16; ACT does sigmoids, yh+bh PSUM evacuation
and output PSUM evacuation; PE matmuls alternate PSUM banks between
consecutive instructions (avoids the +210ns same-bank accumulation bubble).
Output is stored bf16 [O, T]; the host casts/transposes and adds bo.

Sharding: batch b=8 across the 8 cores (one sequence per core); weights
replicated.  The host pre-transposes/casts x and the weights.
"""

from contextlib import ExitStack

import numpy as np
import ml_dtypes

import concourse.bacc as bacc
import concourse.tile as tile
from concourse import mybir
from concourse.bass_utils import run_bass_kernel_spmd

BF16 = ml_dtypes.bfloat16
F32 = mybir.dt.float32
BF = mybir.dt.bfloat16

B, T, D, H, O = 8, 16384, 256, 256, 256
L = 4096          # positions per chunk (power of 2)
NCHUNK = T // L   # 4 full chunks; position T (=16384) handled separately
SUB = 512         # matmul sub-chunk (one PSUM bank at f32)
NSUB = L // SUB   # 8
LMAX = 12         # log2(L)

AluOp = mybir.AluOpType
ActFn = mybir.ActivationFunctionType


def _level_offsets():
    off = {1: 0}
    n = L // 2
    for lvl in range(1, LMAX):
        off[lvl + 1] = off[lvl] + n
        n //= 2
    return off, off[LMAX] + 1


LVL_OFF, LVL_TOTAL = _level_offsets()  # total = 4095


def build_nc():
    nc = bacc.Bacc()

    xt = nc.dram_tensor("xt", [D, T], BF, kind="ExternalInput")
    wzt = nc.dram_tensor("wzt", [D, H], BF, kind="ExternalInput")
    wht = nc.dram_tensor("wht", [D, H], BF, kind="ExternalInput")
    wot = nc.dram_tensor("wot", [H, O], BF, kind="ExternalInput")
    bzp = nc.dram_tensor("bzp", [H, 1], F32, kind="ExternalInput")
    bhb = nc.dram_tensor("bhb", [H, 1], F32, kind="ExternalInput")
    out = nc.dram_tensor("out", [O, T], BF, kind="ExternalOutput")

    with tile.TileContext(nc) as tc, ExitStack() as ctx:
        singles = ctx.enter_context(tc.tile_pool(name="singles", bufs=1))
        ab_pool = ctx.enter_context(tc.tile_pool(name="ab", bufs=2))
        lvl_pool = ctx.enter_context(tc.tile_pool(name="lvl", bufs=1))
        dbuf_pool = ctx.enter_context(tc.tile_pool(name="dbuf", bufs=2))
        tmp_pool = ctx.enter_context(tc.tile_pool(name="tmp", bufs=2))
        zy_pool = ctx.enter_context(tc.tile_pool(name="zy", bufs=3))
        x_pool = ctx.enter_context(tc.tile_pool(name="xp", bufs=3))
        osb_pool = ctx.enter_context(tc.tile_pool(name="osb", bufs=3))
        psum_y = ctx.enter_context(tc.tile_pool(name="psy", bufs=2, space="PSUM"))
        psum_o = ctx.enter_context(tc.tile_pool(name="pso", bufs=2, space="PSUM"))

        # ---- constants ----
        wz_sb, wh_sb, wo_sb = [], [], []
        for k in range(2):
            wzk = singles.tile([128, H], BF, name=f"wzk{k}")
            nc.sync.dma_start(out=wzk, in_=wzt[k * 128:(k + 1) * 128, :])
            wz_sb.append(wzk)
            whk = singles.tile([128, H], BF, name=f"whk{k}")
            nc.sync.dma_start(out=whk, in_=wht[k * 128:(k + 1) * 128, :])
            wh_sb.append(whk)
            wok = singles.tile([128, O], BF, name=f"wok{k}")
            nc.sync.dma_start(out=wok, in_=wot[k * 128:(k + 1) * 128, :])
            wo_sb.append(wok)
        bz_sb, bh_sb = [], []
        for h in range(2):
            pz = singles.tile([128, 1], F32, name=f"bzp{h}")
            nc.sync.dma_start(out=pz, in_=bzp[h * 128:(h + 1) * 128, :])
            bz_sb.append(pz)
            hb = singles.tile([128, 1], F32, name=f"bh{h}")
            nc.sync.dma_start(out=hb, in_=bhb[h * 128:(h + 1) * 128, :])
            bh_sb.append(hb)

        # cross-chunk bookkeeping, halves packed innermost: [128, n, 2]
        tops_A = singles.tile([128, 4, 2], BF, name="topsA")
        tops_B = singles.tile([128, 4, 2], BF, name="topsB")
        otb = singles.tile([128, 4, 2], BF, name="otb")
        spn = singles.tile([128, 4, 2], BF, name="spn")  # 0:t, 1:Ar1, 2:Br1

        abufs = {}

        def emit_phase1(c):
            """DMA x, matmuls, sigmoids, a = 1-z, b = z*h for chunk c."""
            a_buf = ab_pool.tile([128, L, 2], BF, name="a_buf", tag="a")
            b_buf = ab_pool.tile([128, L, 2], BF, name="b_buf", tag="b")
            abufs[c] = (a_buf, b_buf)
            if c == 0:
                nc.vector.memset(a_buf[:, 0:1, :], 1.0)
                nc.vector.memset(b_buf[:, 0:1, :], 0.0)
                subs = [(s * SUB, SUB if s < NSUB - 1 else SUB - 1, s * SUB + 1)
                        for s in range(NSUB)]
            else:
                base = c * L - 1
                subs = [(base + s * SUB, SUB, s * SUB) for s in range(NSUB)]
            for x0, ncols, acol in subs:
                xk = x_pool.tile([128, 2, SUB], BF, name="xk", tag="xk")
                nc.sync.dma_start(
                    out=xk[:, :, :ncols],
                    in_=xt[:, x0:x0 + ncols].rearrange("(k p) n -> p k n", p=128))
                zt = zy_pool.tile([128, SUB, 2], BF, name="zt", tag="zt")
                if c > 0:
                    yhs = zy_pool.tile([128, SUB, 2], BF, name="yhs", tag="yhs")
                for h in range(2):
                    # bank-alternating matmul order: z-k0, h-k0, z-k1, h-k1
                    yz = psum_y.tile([128, SUB], F32, name="yz", tag=f"y{h}")
                    yh = psum_y.tile([128, SUB], F32, name="yh", tag=f"y{h}")
                    for k in range(2):
                        nc.tensor.matmul(yz[:, :ncols],
                                         wz_sb[k][:, h * 128:(h + 1) * 128],
                                         xk[:, k, :ncols],
                                         start=(k == 0), stop=(k == 1))
                        nc.tensor.matmul(yh[:, :ncols],
                                         wh_sb[k][:, h * 128:(h + 1) * 128],
                                         xk[:, k, :ncols],
                                         start=(k == 0), stop=(k == 1))
                    nc.scalar.activation(zt[:, :ncols, h], yz[:, :ncols],
                                         ActFn.Sigmoid,
                                         bias=bz_sb[h][:, 0:1], scale=1.0)
                    if c > 0:
                        nc.scalar.activation(yhs[:, :ncols, h], yh[:, :ncols],
                                             ActFn.Identity,
                                             bias=bh_sb[h][:, 0:1], scale=1.0)
                    else:
                        # head chunk: ACT is the critical engine; compute
                        # b = (yh+bh)*z on the otherwise-idle DVE instead
                        nc.vector.scalar_tensor_tensor(
                            b_buf[:, acol:acol + ncols, h],
                            yh[:, :ncols], bh_sb[h][:, 0:1],
                            zt[:, :ncols, h],
                            op0=AluOp.add, op1=AluOp.mult)
                # a = 1 - z  (tensor_scalar, 4x); b = z*h (TT, 2x)
                nc.vector.tensor_scalar(a_buf[:, acol:acol + ncols, :],
                                        zt[:, :ncols, :], -1.0, 1.0,
                                        op0=AluOp.mult, op1=AluOp.add)
                if c > 0:
                    nc.vector.tensor_tensor(b_buf[:, acol:acol + ncols, :],
                                            yhs[:, :ncols, :], zt[:, :ncols, :],
                                            op=AluOp.mult)

        def emit_up(c):
            a_buf, b_buf = abufs[c]
            Aup = lvl_pool.tile([128, LVL_TOTAL, 2], BF, name="Aup", tag="Au")
            Bup = lvl_pool.tile([128, LVL_TOTAL, 2], BF, name="Bup", tag="Bu")
            for lvl in range(LMAX):
                n = L >> lvl
                m = n // 2
                if lvl == 0:
                    sA, sB = a_buf, b_buf
                else:
                    o = LVL_OFF[lvl]
                    sA = Aup[:, o:o + n, :]
                    sB = Bup[:, o:o + n, :]
                o2 = LVL_OFF[lvl + 1]
                dA = Aup[:, o2:o2 + m, :]
                dB = Bup[:, o2:o2 + m, :]
                A_ev, A_od = sA[:, 0:n:2, :], sA[:, 1:n:2, :]
                B_ev, B_od = sB[:, 0:n:2, :], sB[:, 1:n:2, :]
                nc.vector.tensor_tensor(dA, A_ev, B_ev, op=AluOp.mult)
                tu = tmp_pool.tile([128, L // 2, 2], BF, name="tu", tag="tmp")
                nc.vector.tensor_tensor(tu[:, :m, :], B_ev, A_od, op=AluOp.mult)
                nc.vector.tensor_tensor(dB, tu[:, :m, :], B_od, op=AluOp.add)
            return Aup, Bup

        def emit_spine(c, Aup, Bup):
            o12 = LVL_OFF[LMAX]
            EA = tops_A[:, c:c + 1, :]
            EB = tops_B[:, c:c + 1, :]
            nc.vector.tensor_copy(EA, Aup[:, o12:o12 + 1, :])
            nc.vector.tensor_copy(EB, Bup[:, o12:o12 + 1, :])
            t = spn[:, 0:1, :]
            if c == 0:
                nc.vector.tensor_copy(otb[:, 0:1, :], EB)
            elif c == 1:
                # otb1 = B0*A1 + B1
                nc.vector.tensor_tensor(t, tops_B[:, 0:1, :], EA, op=AluOp.mult)
                nc.vector.tensor_tensor(otb[:, 1:2, :], t, EB, op=AluOp.add)
            elif c == 2:
                # otb2 = otb1*A2 + B2
                nc.vector.tensor_tensor(t, otb[:, 1:2, :], EA, op=AluOp.mult)
                nc.vector.tensor_tensor(otb[:, 2:3, :], t, EB, op=AluOp.add)
            elif c == 3:
                # Br1 = B2*A3 + B3 ; Ar1 = A2*B2 ; otb3 = otb1*Ar1 + Br1
                nc.vector.tensor_tensor(t, tops_B[:, 2:3, :], EA, op=AluOp.mult)
                nc.vector.tensor_tensor(spn[:, 2:3, :], t, EB, op=AluOp.add)
                nc.vector.tensor_tensor(spn[:, 1:2, :], tops_A[:, 2:3, :],
                                        tops_B[:, 2:3, :], op=AluOp.mult)
                nc.vector.tensor_tensor(t, otb[:, 1:2, :], spn[:, 1:2, :],
                                        op=AluOp.mult)
                nc.vector.tensor_tensor(otb[:, 3:4, :], t, spn[:, 2:3, :],
                                        op=AluOp.add)

        def emit_down(c, Aup, Bup):
            a_buf, b_buf = abufs.pop(c)
            dbuf = dbuf_pool.tile([128, L + 1, 2], BF, name="dbuf", tag="d")
            if c == 0:
                nc.vector.memset(dbuf[:, 0:1, :], 0.0)
            else:
                nc.vector.tensor_copy(dbuf[:, 0:1, :], otb[:, c - 1:c, :])
            nc.vector.tensor_copy(dbuf[:, L:L + 1, :], otb[:, c:c + 1, :])
            for lvl in range(LMAX - 1, -1, -1):
                n = L >> lvl
                cnt = n // 2
                step = 1 << (lvl + 1)
                if lvl == 0:
                    A_src, B_src = a_buf, b_buf
                else:
                    o = LVL_OFF[lvl]
                    A_src = Aup[:, o:o + n, :]
                    B_src = Bup[:, o:o + n, :]
                A_ev = A_src[:, 0:n:2, :]
                B_ev = B_src[:, 0:n:2, :]
                Lh = dbuf[:, 0:L:step, :]
                Wt = dbuf[:, (1 << lvl):L:step, :]
                td = tmp_pool.tile([128, L // 2, 2], BF, name="td", tag="tmp")
                nc.vector.tensor_tensor(td[:, :cnt, :], Lh, A_ev, op=AluOp.mult)
                nc.vector.tensor_tensor(Wt, td[:, :cnt, :], B_ev, op=AluOp.add)
            return dbuf

        def emit_out(c, dbuf):
            obase = c * L - 1
            for s in range(NSUB):
                col0 = s * SUB
                po = psum_o.tile([128, 2, SUB], F32, name="po", tag="po")
                # bank-alternating: oh0-k0, oh1-k0, oh0-k1, oh1-k1
                for k in range(2):
                    for oh in range(2):
                        nc.tensor.matmul(po[:, oh, :],
                                         wo_sb[k][:, oh * 128:(oh + 1) * 128],
                                         dbuf[:, 1 + col0:1 + col0 + SUB, k],
                                         start=(k == 0), stop=(k == 1))
                osb = osb_pool.tile([128, 2, SUB], BF, name="osb", tag="osb")
                nc.scalar.copy(osb, po)
                skip = 1 if (c == 0 and s == 0) else 0
                dst = out[:, obase + col0 + skip:obase + col0 + SUB]
                nc.sync.dma_start(
                    out=dst.rearrange("(two p) n -> p two n", p=128),
                    in_=osb[:, :, skip:])

        # ---- software-pipelined emission ----
        emit_phase1(0)
        for c in range(NCHUNK):
            Aup_c, Bup_c = emit_up(c)
            emit_spine(c, Aup_c, Bup_c)
            if c + 1 < NCHUNK:
                emit_phase1(c + 1)
            dbuf_c = emit_down(c, Aup_c, Bup_c)
            emit_out(c, dbuf_c)
            last_dbuf = dbuf_c

        # ---- final position T: out[T-1] = S[T-1]*a_T + b_T fed to Wo ----
        xl = singles.tile([128, 2, 1], BF, name="xl")
        nc.sync.dma_start(out=xl,
                          in_=xt[:, T - 1:T].rearrange("(k p) n -> p k n", p=128))
        zl = singles.tile([128, 1, 2], BF, name="zl")
        yl = singles.tile([128, 1, 2], BF, name="yl")
        for h in range(2):
            yzl = psum_y.tile([128, SUB], F32, name="yzl", tag=f"y{h}")[:, 0:1]
            yhl = psum_y.tile([128, SUB], F32, name="yhl", tag=f"y{h}")[:, 0:1]
            for k in range(2):
                nc.tensor.matmul(yzl, wz_sb[k][:, h * 128:(h + 1) * 128],
                                 xl[:, k, :], start=(k == 0), stop=(k == 1))
                nc.tensor.matmul(yhl, wh_sb[k][:, h * 128:(h + 1) * 128],
                                 xl[:, k, :], start=(k == 0), stop=(k == 1))
            nc.scalar.activation(zl[:, :, h], yzl, ActFn.Sigmoid,
                                 bias=bz_sb[h][:, 0:1], scale=1.0)
            nc.scalar.activation(yl[:, :, h], yhl, ActFn.Identity,
                                 bias=bh_sb[h][:, 0:1], scale=1.0)
        al = singles.tile([128, 1, 2], BF, name="al")
        bl = singles.tile([128, 1, 2], BF, name="bl")
        nc.vector.tensor_scalar(al, zl, -1.0, 1.0,
                                op0=AluOp.mult, op1=AluOp.add)
        nc.vector.tensor_tensor(bl, yl, zl, op=AluOp.mult)
        dl = singles.tile([128, 1, 2], BF, name="dl")
        sl = singles.tile([128, 1, 2], BF, name="sl")
        nc.vector.tensor_tensor(dl, last_dbuf[:, L:L + 1, :], al, op=AluOp.mult)
        nc.vector.tensor_tensor(sl, dl, bl, op=AluOp.add)
        pol = psum_o.tile([128, 2, SUB], F32, name="pol", tag="po")[:, :, 0:1]
        for k in range(2):
            for oh in range(2):
                nc.tensor.matmul(pol[:, oh, :],
                                 wo_sb[k][:, oh * 128:(oh + 1) * 128],
                                 sl[:, :, k], start=(k == 0), stop=(k == 1))
        osl = singles.tile([128, 2, 1], BF, name="osl")
        nc.scalar.copy(osl, pol)
        nc.sync.dma_start(
            out=out[:, T - 1:T].rearrange("(two p) n -> p two n", p=128),
            in_=osl)

    nc.compile()
    return nc


_NC_CACHE = {}


def _get_nc():
    if "nc" not in _NC_CACHE:
        _NC_CACHE["nc"] = build_nc()
    return _NC_CACHE["nc"]


def _prepare_in_maps(xs, Wz, bz, Wh, bh, Wo, bo):
    xs = np.asarray(xs, np.float32)
    Wz = np.asarray(Wz, np.float32)
    bz = np.asarray(bz, np.float32)
    Wh = np.asarray(Wh, np.float32)
    bh = np.asarray(bh, np.float32)
    Wo = np.asarray(Wo, np.float32)

    wzt = np.ascontiguousarray(Wz.T).astype(BF16)
    wht = np.ascontiguousarray(Wh.T).astype(BF16)
    wot = np.ascontiguousarray(Wo.T).astype(BF16)
    bzp = np.ascontiguousarray(bz.reshape(H, 1))
    bhb = np.ascontiguousarray(bh.reshape(H, 1))

    in_maps = []
    for i in range(B):
        xti = np.ascontiguousarray(xs[i].T).astype(BF16)
        in_maps.append({
            "xt": xti, "wzt": wzt, "wht": wht, "wot": wot,
            "bzp": bzp, "bhb": bhb,
        })
    return in_maps


def _assemble(res, bo):
    bo = np.asarray(bo, np.float32)
    return np.stack([np.asarray(res.results[i]["out"]).astype(np.float32).T + bo
                     for i in range(B)], axis=0)


def run_traced(xs, Wz, bz, Wh, bh, Wo, bo, trace=True):
    in_maps = _prepare_in_maps(xs, Wz, bz, Wh, bh, Wo, bo)
    res = run_bass_kernel_spmd(_get_nc(), in_maps, core_ids=list(range(B)),
                               trace=trace)
    return _assemble(res, bo), res


def kernel(xs, Wz, bz, Wh, bh, Wo, bo):
    in_maps = _prepare_in_maps(xs, Wz, bz, Wh, bh, Wo, bo)
    res = run_bass_kernel_spmd(_get_nc(), in_maps, core_ids=list(range(B)))
    return _assemble(res, bo)
